# revision 24
# baseline (speedup 1.0000x reference)
"""Trainium2 Bass kernel for causal multi-head attention with RoPE.

Problem: B=2, S=2048, DIM=1024, 16 heads, head_dim=64.
  q = x @ Wq.T ; k = x @ Wk.T ; v = x @ Wv.T        (torch Linear convention)
  q, k = rope(q), rope(k)                            (Llama interleaved pairs)
  y = softmax(causal(q k^T / 8)) v @ Wo.T

Sharding (8 cores): data-parallel over batch (2) x tensor-parallel over
head groups (4 heads per core).  Wq/Wk/Wv row-sharded, Wo column-sharded;
the host sums the 4 partial outputs per batch.

v2: one fused pipeline instead of serial phases.  The exp stream on
ScalarE (~105us incl. per-instr overhead) and the PE column stream
(~112us) are the two floors; the program interleaves them so neither
engine idles: attention scores+exp start ~8us in, and all projection /
V-projection / O-projection matmuls are woven into the attention loop
as PE filler, which also keeps the PE busy enough that the HAM clock
gate stays at 8/8 (the baseline lost ~35us to 1.2GHz throttling).

PSUM budget (8 banks x 2KB):
  ST   1 tag x 2 bufs x [128,512] f32 = 2 banks  (score^T chunks; the
       single-buffering per head paces the PE to the ACT exp stream)
  OB   2 bufs x [65,1024] f32         = 4 banks  (PV accum + ones-row
       denominator, one per head)
  pp   2 bufs x [128,512] f32         = 2 banks  (proj + O-proj chunks)

Per (qtile, pair) unit j-loop: scores h0/h1 issued adjacently (PE row
groups 0 and 64 -> concurrent row-tiled matmuls), exp per 512-col chunk
on ACT, inline PV at lag 2, normalization per 512-col half as soon as
its j's complete (reciprocal_approx_fast on the PSUM ones-row, GPSIMD
partition-broadcast, one DVE multiply into Z^T fp16).  O-projection
chunks chase the normalized halves and DMA out fp16.
"""

import os
import sys

sys.path.insert(0, "/opt/trn_rl_repo")

from collections import deque

import numpy as np

import concourse.bass as bass
import concourse.mybir as mybir
import concourse.tile as tile
from concourse import bacc
from concourse.bass_utils import run_bass_kernel_spmd

F16 = mybir.dt.float16
F32 = mybir.dt.float32

DIM = 1024
NUM_HEADS = 16
HEAD_DIM = 64
B = 2
GROUPS = 4                   # head groups (tensor parallel)
HPG = NUM_HEADS // GROUPS    # heads per group = 4
FG = HPG * HEAD_DIM          # features per group = 256
THETA = 10000.0


def build_program(S=2048):
    from contextlib import ExitStack

    nc = bacc.Bacc(None, target_bir_lowering=False)
    NT = S // 128                 # token blocks
    QTILE = 512
    NQT = S // QTILE

    # all inputs host-prearranged to [partition, ...] dense layouts so every
    # DMA is a single-stage contiguous transfer (no on-the-fly rearrange)
    xt_d = nc.declare_dram_parameter("xt", [128, 8, S], F16, isOutput=False)
    wq_d = nc.declare_dram_parameter("wqt", [128, 8, FG], F16, isOutput=False)
    wk_d = nc.declare_dram_parameter("wkt", [128, 8, FG], F16, isOutput=False)
    wv_d = nc.declare_dram_parameter("wvt", [128, 8, FG], F16, isOutput=False)
    wo_d = nc.declare_dram_parameter("wot", [128, 2, DIM], F16, isOutput=False)
    cos_d = nc.declare_dram_parameter("cos", [128, S], F16, isOutput=False)
    sin_d = nc.declare_dram_parameter("sins", [128, S], F16, isOutput=False)
    mask_d = nc.declare_dram_parameter("mask", [128, 128], F16, isOutput=False)
    # fp16 partial output [partition, dchunk, token]; host reassembles
    yt_d = nc.declare_dram_parameter("yt", [128, 8, S], F16, isOutput=True)

    Exp = mybir.ActivationFunctionType.Exp

    with tile.TileContext(nc) as tc:
        with ExitStack() as ctx:
            consts = ctx.enter_context(tc.tile_pool(name="consts", bufs=1))

            # ---- constants / persistent SBUF ----
            dummy_sb = consts.tile([128, 512], F16)
            nc.vector.memset(dummy_sb[:], 1.0)
            wk_sb = consts.tile([128, 8, FG], F16)
            nc.sync.dma_start(wk_sb[:], wk_d[:])
            xt_sb = consts.tile([128, 8, S], F16)
            nc.sync.dma_start(xt_sb[:, 0:4, 0:512], xt_d[:, 0:4, 0:512])
            nc.sync.dma_start(xt_sb[:, 4:8, 0:512], xt_d[:, 4:8, 0:512])
            wq_sb = consts.tile([128, 8, FG], F16)
            nc.sync.dma_start(wq_sb[:], wq_d[:])
            cos_t = consts.tile([128, S], F16)
            nc.sync.dma_start(cos_t[:, 0:512], cos_d[:, 0:512])
            sin_t = consts.tile([128, S], F16)
            nc.sync.dma_start(sin_t[:, 0:512], sin_d[:, 0:512])
            mask_t = consts.tile([128, 128], F16)
            nc.sync.dma_start(mask_t[:], mask_d[:])
            nc.sync.dma_start(cos_t[:, 512:S], cos_d[:, 512:S])
            nc.sync.dma_start(sin_t[:, 512:S], sin_d[:, 512:S])
            wv_sb = consts.tile([128, 8, FG], F16)
            nc.sync.dma_start(wv_sb[:], wv_d[:])
            for t in range(1, 3):
                c0, c1 = t * 512, (t + 1) * 512
                nc.sync.dma_start(xt_sb[:, :, c0:c1], xt_d[:, :, c0:c1])
            wo_sb = consts.tile([128, 2, DIM], F16)
            nc.sync.dma_start(wo_sb[:], wo_d[:])
            nc.sync.dma_start(xt_sb[:, :, 1536:2048], xt_d[:, :, 1536:2048])

            qt_sb = consts.tile([128, 2, S], F16)
            kt_sb = consts.tile([128, 2, S], F16)
            vaug = consts.tile([128, NT, HPG * 65], F16)
            zt_sb = consts.tile([128, 2, S], F16)
            nc.vector.memset(vaug[:], 1.0)
            mask2 = consts.tile([128, 2, 128], F16)
            nc.vector.tensor_copy(mask2[:, 0, :], mask_t[:])
            nc.vector.tensor_copy(mask2[:, 1, :], mask_t[:])

            # ---- pools ----
            stp = ctx.enter_context(tc.tile_pool(name="stp", bufs=2, space="PSUM"))
            obp = ctx.enter_context(tc.tile_pool(name="obp", bufs=3, space="PSUM"))
            pp = ctx.enter_context(tc.tile_pool(name="pp", bufs=1, space="PSUM"))
            ptp = ctx.enter_context(tc.tile_pool(name="ptp", bufs=10))
            rp = ctx.enter_context(tc.tile_pool(name="rope", bufs=8))
            smp = ctx.enter_context(tc.tile_pool(name="smp", bufs=4))
            bcp = ctx.enter_context(tc.tile_pool(name="bcp", bufs=4))
            yp = ctx.enter_context(tc.tile_pool(name="ysb", bufs=4))

            # HAM warm-up: ~13us of dummy matmuls during the DMA window so
            # the PE clock gate is at 8/8 when real work starts; the final
            # exp read also pulls in the ACT exp table set early.
            dummy_ps = stp.tile([128, 2, 512], F32, tag="st", name="dummy_ps")
            for _ in range(16):
                nc.tensor.matmul(
                    dummy_ps[:, 0, :],
                    lhsT=dummy_sb[:, 0:128],
                    rhs=dummy_sb[:],
                    start=True,
                    stop=True,
                    skip_group_check=True,
                )
            warm2 = consts.tile([1, 2], F16)
            nc.scalar.activation(warm2[:], dummy_ps[0:1, 0, 0:2], Exp, scale=1.0)

            fillers = deque()

            def weave(n):
                for _ in range(n):
                    if fillers:
                        fillers.popleft()()

            # ================= projection machinery =================
            def make_proj_chunk(wsb, dest, c, t):
                def emit():
                    lo, hi = t * 512, (t + 1) * 512
                    psq = pp.tile([128, 512], F32, tag="proj", name=f"ps_{c}_{t}")
                    for k in range(8):
                        nc.tensor.matmul(
                            psq[:],
                            lhsT=wsb[:, k, c * 128:(c + 1) * 128],
                            rhs=xt_sb[:, k, lo:hi],
                            start=(k == 0),
                            stop=(k == 7),
                        )
                    qc = rp.tile([128, 512], F16, tag="rope")
                    nc.vector.tensor_copy(qc[:], psq[:])  # fp32->fp16 cast
                    t1 = rp.tile([128, 512], F16, tag="rope")
                    nc.vector.tensor_mul(t1[:], qc[:], cos_t[:, lo:hi])
                    rot = rp.tile([128, 512], F16, tag="rope")
                    for qq in range(4):
                        srcp = (qq ^ 1) * 32
                        nc.vector.tensor_copy(
                            rot[qq * 32:(qq + 1) * 32, :], qc[srcp:srcp + 32, :]
                        )
                    t2 = rp.tile([128, 512], F16, tag="rope")
                    nc.vector.tensor_mul(t2[:], rot[:], sin_t[:, lo:hi])
                    nc.vector.tensor_add(dest[:, c, lo:hi], t1[:], t2[:])
                return emit

            def make_vproj_block(tb):
                def emit():
                    psv = pp.tile([128, 512], F32, tag="proj", name=f"psv_{tb}")
                    for k in range(8):
                        nc.tensor.matmul(
                            psv[:, 0:FG],
                            lhsT=xt_sb[:, k, tb * 128:(tb + 1) * 128],
                            rhs=wv_sb[:, k, :],
                            start=(k == 0),
                            stop=(k == 7),
                        )
                    nc.vector.tensor_copy(
                        vaug[:, tb, :].rearrange("p (h c) -> p h c", c=65)[:, :, 0:64],
                        psv[:, 0:FG].rearrange("p (h d) -> p h d", d=64),
                    )
                return emit

            # ================= O-projection machinery =================
            yt_r = yt_d[:]

            def make_oproj_chunk(dchunk, scol, cast_act=False, pool=None):
                def emit():
                    psy = (pool or pp).tile(
                        [128, 512], F32,
                        tag="ob" if pool is not None else "proj",
                        name=f"psy_{dchunk}_{scol}",
                    )
                    for c2 in range(2):
                        nc.tensor.matmul(
                            psy[:],
                            lhsT=wo_sb[:, c2, dchunk * 128:(dchunk + 1) * 128],
                            rhs=zt_sb[:, c2, scol:scol + 512],
                            start=(c2 == 0),
                            stop=(c2 == 1),
                        )
                    ytile = yp.tile(
                        [128, 512], F16, tag="y", name=f"y_{dchunk}_{scol}"
                    )
                    if cast_act:
                        nc.scalar.copy(ytile[:], psy[:])
                    else:
                        nc.vector.tensor_copy(ytile[:], psy[:])
                    nc.sync.dma_start(yt_r[:, dchunk, scol:scol + 512], ytile[:])
                return emit

            def owindow_chunks(scol_base, cast_act=False):
                out = []
                for dchunk in range(8):
                    out.append(make_oproj_chunk(dchunk, scol_base, cast_act))
                    out.append(make_oproj_chunk(dchunk, scol_base + 512, cast_act))
                return out

            # ================= attention unit =================
            # unit (qt, p): heads hh=0 rows 0:64, hh=1 rows 64:128, slot p.
            def emit_unit(qt, p, nf_per_j):
                qlo = qt * QTILE
                jmax = (qlo + QTILE) // 128
                pts = {}
                obs = {}
                for hh in range(2):
                    obs[hh] = obp.tile(
                        [65, QTILE], F32, tag="ob", name=f"ob_{qt}_{p}_{hh}"
                    )

                def pv(j):
                    pt, a0, w = pts.pop(j)
                    for hh in range(2):
                        hg = p * 2 + hh
                        nc.tensor.matmul(
                            obs[hh][:, a0:a0 + w],
                            lhsT=vaug[:, j, hg * 65:(hg + 1) * 65],
                            rhs=pt[:, hh, 0:w],
                            start=(j == 0),
                            stop=(j == jmax - 1),
                            skip_group_check=True,
                        )

                def normalize(hh):
                    cols = slice(qlo, qlo + QTILE)
                    lrow = smp.tile([1, QTILE], F32, tag="lrow")
                    nc.vector.tensor_copy(lrow[:], obs[hh][64:65, :])
                    rcp = smp.tile([1, QTILE], F32, tag="rcp")
                    nc.vector.reciprocal_approx_fast(rcp[:], lrow[:])
                    bc = bcp.tile([64, QTILE], F32, tag="bc")
                    nc.gpsimd.partition_broadcast(bc[:], rcp[:])
                    nc.vector.tensor_mul(
                        zt_sb[hh * 64:(hh + 1) * 64, p, cols], obs[hh][0:64, :], bc[:]
                    )

                for j in range(jmax):
                    qs = max(qlo, j * 128)
                    a0 = qs - qlo
                    w = QTILE - a0
                    st = stp.tile([128, 2, 512], F32, tag="st", name=f"st_{qt}_{p}_{j}")
                    for hh in range(2):
                        base = 64 * hh
                        nc.tensor.matmul(
                            st[:, hh, 0:w],
                            lhsT=kt_sb[base:base + 64, p, j * 128:(j + 1) * 128],
                            rhs=qt_sb[base:base + 64, p, qs:qs + w],
                            start=True,
                            stop=True,
                        )
                    pt = ptp.tile([128, 2, 512], F16, tag="pt")
                    nc.scalar.activation(pt[:, :, 0:w], st[:, :, 0:w], Exp, scale=0.125)
                    if j * 128 >= qlo:
                        nc.gpsimd.tensor_mul(pt[:, :, 0:128], pt[:, :, 0:128], mask2[:])
                    pts[j] = (pt, a0, w)
                    if j >= 2:
                        pv(j - 2)
                    weave(nf_per_j)
                if jmax >= 2:
                    pv(jmax - 2)
                pv(jmax - 1)
                normalize(0)
                normalize(1)

            def owindow(qlo, cast_act=False, pool=None):
                return [make_oproj_chunk(d, qlo, cast_act, pool) for d in range(8)]

            # ================= schedule =================
            # prologue: just enough K/Q/V for (qt0, p0)
            make_proj_chunk(wk_sb, kt_sb, 0, 0)()
            make_proj_chunk(wq_sb, qt_sb, 0, 0)()
            make_vproj_block(0)()
            # filler order respects downstream deps (qtiles are 512 cols;
            # K chunk t covers k-blocks 4t..4t+3; V block tb = k-block tb)
            fillers.append(make_vproj_block(1))
            fillers.append(make_vproj_block(2))
            fillers.append(make_vproj_block(3))
            fillers.append(make_proj_chunk(wq_sb, qt_sb, 1, 0))   # qt0p1
            fillers.append(make_proj_chunk(wk_sb, kt_sb, 1, 0))
            fillers.append(make_proj_chunk(wk_sb, kt_sb, 0, 1))   # qt1
            fillers.append(make_proj_chunk(wq_sb, qt_sb, 0, 1))
            for tb in range(4, 8):
                fillers.append(make_vproj_block(tb))
            fillers.append(make_proj_chunk(wk_sb, kt_sb, 1, 1))
            fillers.append(make_proj_chunk(wq_sb, qt_sb, 1, 1))
            fillers.append(make_proj_chunk(wk_sb, kt_sb, 0, 2))   # qt2
            fillers.append(make_proj_chunk(wq_sb, qt_sb, 0, 2))
            for tb in range(8, 12):
                fillers.append(make_vproj_block(tb))
            fillers.append(make_proj_chunk(wk_sb, kt_sb, 1, 2))
            fillers.append(make_proj_chunk(wq_sb, qt_sb, 1, 2))
            fillers.append(make_proj_chunk(wk_sb, kt_sb, 0, 3))   # qt3
            fillers.append(make_proj_chunk(wq_sb, qt_sb, 0, 3))
            fillers.append(make_proj_chunk(wk_sb, kt_sb, 1, 3))
            fillers.append(make_proj_chunk(wq_sb, qt_sb, 1, 3))

            emit_unit(0, 0, nf_per_j=3)
            emit_unit(0, 1, nf_per_j=3)
            w0 = owindow(0)
            fillers.extend(w0[0:4])
            emit_unit(1, 0, nf_per_j=2)
            fillers.extend(w0[4:8])
            emit_unit(1, 1, nf_per_j=2)
            w1 = owindow(512)
            fillers.extend(w1[0:4])
            emit_unit(2, 0, nf_per_j=1)
            fillers.extend(w1[4:8])
            emit_unit(2, 1, nf_per_j=1)
            for tb in range(12, 16):
                fillers.append(make_vproj_block(tb))
            w2 = owindow(1024)
            fillers.extend(w2[0:4])
            emit_unit(3, 0, nf_per_j=1)
            fillers.extend(w2[4:8])
            emit_unit(3, 1, nf_per_j=1)
            # tail: leftovers + final O window (casts on ACT, idle then)
            while fillers:
                fillers.popleft()()
            for fn in owindow(1536, cast_act=True, pool=obp):
                fn()

    nc.compile()
    return nc


def _host_inputs(x, Wq, Wk, Wv, Wo, S):
    """Per-core input maps (host-side sharding + layout prep)."""
    # de-interleave RoPE pairs within each head: (2i, 2i+1) -> (i, i+32)
    perm = np.concatenate([np.arange(0, HEAD_DIM, 2), np.arange(1, HEAD_DIM, 2)])
    rp = (np.arange(HPG)[:, None] * HEAD_DIM + perm[None, :]).reshape(-1)

    half = HEAD_DIM // 2
    inv_freq = THETA ** (-np.arange(half, dtype=np.float64) * 2.0 / HEAD_DIM)
    ang = np.arange(S, dtype=np.float64)[None, :] * inv_freq[:, None]  # [32, S]
    cos32 = np.cos(ang)
    sin32 = np.sin(ang)
    cos128 = np.tile(cos32, (4, 1)).astype(np.float16)
    sins128 = np.concatenate([-sin32, sin32, -sin32, sin32], axis=0).astype(np.float16)
    mask = (np.arange(128)[None, :] >= np.arange(128)[:, None]).astype(np.float16)

    def part3(a, c):
        # [DIM_like, F] -> [128, c, F] partition-major dense
        return np.ascontiguousarray(
            a.reshape(c, 128, a.shape[1]).transpose(1, 0, 2)
        ).astype(np.float16)

    in_maps = []
    for core in range(B * GROUPS):
        b, g = divmod(core, GROUPS)
        sl = slice(g * FG, (g + 1) * FG)
        in_maps.append(
            dict(
                xt=part3(x[b].T.astype(np.float32), 8),
                wqt=part3(Wq[sl][rp].T, 8),
                wkt=part3(Wk[sl][rp].T, 8),
                wvt=part3(Wv[sl].T, 8),
                wot=part3(Wo[:, sl].T, 2),
                cos=cos128,
                sins=sins128,
                mask=mask,
            )
        )
    return in_maps


def _install_ntff_hook():
    """Provide antenv.axon_hooks if the image lacks it (NTFF profiling
    under axon; mirrors trn_agent_boot._ntff_profile_via_ctypes)."""
    try:
        from antenv.axon_hooks import get_axon_ntff_profile_hook  # noqa: F401
        return
    except ImportError:
        pass
    import contextlib
    import ctypes
    import types

    so_path = "/opt/axon/libaxon_pjrt.so"
    if not os.path.exists(so_path):
        return
    lib = ctypes.CDLL(so_path)
    if not hasattr(lib, "axon_start_nrt_profile"):
        return
    lib.axon_start_nrt_profile.argtypes = [
        ctypes.POINTER(ctypes.c_int64),
        ctypes.c_size_t,
    ]
    lib.axon_start_nrt_profile.restype = ctypes.c_int64
    lib.axon_stop_nrt_profile.argtypes = [ctypes.c_char_p]
    lib.axon_stop_nrt_profile.restype = ctypes.c_int64

    @contextlib.contextmanager
    def _hook(output_dir, device_ids):
        import jax

        jax.devices()
        if device_ids:
            ids = (ctypes.c_int64 * len(device_ids))(*device_ids)
            rc = lib.axon_start_nrt_profile(ids, len(device_ids))
        else:
            rc = lib.axon_start_nrt_profile(None, 0)
        if rc != 0:
            raise RuntimeError(f"axon_start_nrt_profile rc={rc}")
        try:
            yield
        finally:
            n = lib.axon_stop_nrt_profile(str(output_dir).encode())
            print(f"profile: {n} file(s) written to {output_dir}")

    mod = types.ModuleType("antenv.axon_hooks")
    _state = {"hook": _hook}
    mod.get_axon_ntff_profile_hook = lambda: _state["hook"]
    mod.set_axon_ntff_profile_hook = lambda h: _state.__setitem__("hook", h)
    import antenv

    antenv.axon_hooks = mod
    sys.modules["antenv.axon_hooks"] = mod


_NC_CACHE = {}


def _get_nc(S):
    if S not in _NC_CACHE:
        _NC_CACHE[S] = build_program(S)
    return _NC_CACHE[S]


def kernel(x, Wq, Wk, Wv, Wo, _trace=False, _tmpdir=None):
    x = np.asarray(x, dtype=np.float32)
    Wq = np.asarray(Wq, dtype=np.float32)
    Wk = np.asarray(Wk, dtype=np.float32)
    Wv = np.asarray(Wv, dtype=np.float32)
    Wo = np.asarray(Wo, dtype=np.float32)
    S = x.shape[1]

    if _trace:
        _install_ntff_hook()
    nc = _get_nc(S)
    in_maps = _host_inputs(x, Wq, Wk, Wv, Wo, S)
    res = run_bass_kernel_spmd(
        nc, in_maps, core_ids=list(range(8)), trace=_trace, tmpdir=_tmpdir
    )
    # yt arrives as [128, 8, S]; reassemble to [DIM, S] then transpose
    yts = [
        res.results[c]["yt"].astype(np.float32).transpose(1, 0, 2).reshape(DIM, S)
        for c in range(8)
    ]
    y = np.stack(
        [sum(yts[b * GROUPS + g] for g in range(GROUPS)).T for b in range(B)]
    ).astype(np.float32)
    if _trace:
        kernel.last_results = res
    return y


# revision 25
# speedup vs baseline: 1.2119x; 1.2119x over previous
"""Trainium2 Bass kernel for causal multi-head attention with RoPE.

Problem: B=2, S=2048, DIM=1024, 16 heads, head_dim=64.
  q = x @ Wq.T ; k = x @ Wk.T ; v = x @ Wv.T        (torch Linear convention)
  q, k = rope(q), rope(k)                            (Llama interleaved pairs)
  y = softmax(causal(q k^T / 8)) v @ Wo.T

Sharding (8 cores): data-parallel over batch (2) x tensor-parallel over
head groups (4 heads per core).  Wq/Wk/Wv row-sharded, Wo column-sharded;
the host sums the 4 partial outputs per batch.

v2: one fused pipeline instead of serial phases.  The exp stream on
ScalarE (~105us incl. per-instr overhead) and the PE column stream
(~112us) are the two floors; the program interleaves them so neither
engine idles: attention scores+exp start ~8us in, and all projection /
V-projection / O-projection matmuls are woven into the attention loop
as PE filler, which also keeps the PE busy enough that the HAM clock
gate stays at 8/8 (the baseline lost ~35us to 1.2GHz throttling).

PSUM budget (8 banks x 2KB):
  ST   1 tag x 2 bufs x [128,512] f32 = 2 banks  (score^T chunks; the
       single-buffering per head paces the PE to the ACT exp stream)
  OB   2 bufs x [65,1024] f32         = 4 banks  (PV accum + ones-row
       denominator, one per head)
  pp   2 bufs x [128,512] f32         = 2 banks  (proj + O-proj chunks)

Per (qtile, pair) unit j-loop: scores h0/h1 issued adjacently (PE row
groups 0 and 64 -> concurrent row-tiled matmuls), exp per 512-col chunk
on ACT, inline PV at lag 2, normalization per 512-col half as soon as
its j's complete (reciprocal_approx_fast on the PSUM ones-row, GPSIMD
partition-broadcast, one DVE multiply into Z^T fp16).  O-projection
chunks chase the normalized halves and DMA out fp16.
"""

import os
import sys

sys.path.insert(0, "/opt/trn_rl_repo")

from collections import deque

import numpy as np

import concourse.bass as bass
import concourse.mybir as mybir
import concourse.tile as tile
from concourse import bacc
from concourse.bass_utils import run_bass_kernel_spmd

F16 = mybir.dt.float16
F32 = mybir.dt.float32

DIM = 1024
NUM_HEADS = 16
HEAD_DIM = 64
B = 2
GROUPS = 4                   # head groups (tensor parallel)
HPG = NUM_HEADS // GROUPS    # heads per group = 4
FG = HPG * HEAD_DIM          # features per group = 256
THETA = 10000.0


def build_program(S=2048):
    from contextlib import ExitStack

    nc = bacc.Bacc(None, target_bir_lowering=False)
    NT = S // 128                 # token blocks
    QTILE = 512
    NQT = S // QTILE

    # all inputs host-prearranged to [partition, ...] dense layouts so every
    # DMA is a single-stage contiguous transfer (no on-the-fly rearrange)
    xt_d = nc.declare_dram_parameter("xt", [128, 8, S], F16, isOutput=False)
    wq_d = nc.declare_dram_parameter("wqt", [128, 8, FG], F16, isOutput=False)
    wk_d = nc.declare_dram_parameter("wkt", [128, 8, FG], F16, isOutput=False)
    wv_d = nc.declare_dram_parameter("wvt", [128, 8, FG], F16, isOutput=False)
    wo_d = nc.declare_dram_parameter("wot", [128, 2, DIM], F16, isOutput=False)
    cos_d = nc.declare_dram_parameter("cos", [128, S], F16, isOutput=False)
    sin_d = nc.declare_dram_parameter("sins", [128, S], F16, isOutput=False)
    mask_d = nc.declare_dram_parameter("mask", [128, 128], F16, isOutput=False)
    # fp16 partial output [partition, dchunk, token]; host reassembles
    yt_d = nc.declare_dram_parameter("yt", [128, 8, S], F16, isOutput=True)

    Exp = mybir.ActivationFunctionType.Exp

    with tile.TileContext(nc) as tc:
        with ExitStack() as ctx:
            consts = ctx.enter_context(tc.tile_pool(name="consts", bufs=1))

            # ---- constants / persistent SBUF ----
            dummy_sb = consts.tile([128, 512], F16)
            nc.vector.memset(dummy_sb[:], 1.0)
            wk_sb = consts.tile([128, 8, FG], F16)
            nc.sync.dma_start(wk_sb[:], wk_d[:])
            xt_sb = consts.tile([128, 8, S], F16)
            nc.sync.dma_start(xt_sb[:, 0:4, 0:512], xt_d[:, 0:4, 0:512])
            nc.sync.dma_start(xt_sb[:, 4:8, 0:512], xt_d[:, 4:8, 0:512])
            wq_sb = consts.tile([128, 8, FG], F16)
            nc.sync.dma_start(wq_sb[:], wq_d[:])
            cos_t = consts.tile([128, S], F16)
            nc.sync.dma_start(cos_t[:, 0:512], cos_d[:, 0:512])
            sin_t = consts.tile([128, S], F16)
            nc.sync.dma_start(sin_t[:, 0:512], sin_d[:, 0:512])
            mask_t = consts.tile([128, 128], F16)
            nc.sync.dma_start(mask_t[:], mask_d[:])
            nc.sync.dma_start(cos_t[:, 512:S], cos_d[:, 512:S])
            nc.sync.dma_start(sin_t[:, 512:S], sin_d[:, 512:S])
            wv_sb = consts.tile([128, 8, FG], F16)
            nc.sync.dma_start(wv_sb[:], wv_d[:])
            for t in range(1, 3):
                c0, c1 = t * 512, (t + 1) * 512
                nc.sync.dma_start(xt_sb[:, :, c0:c1], xt_d[:, :, c0:c1])
            wo_sb = consts.tile([128, 2, DIM], F16)
            nc.sync.dma_start(wo_sb[:], wo_d[:])
            nc.sync.dma_start(xt_sb[:, :, 1536:2048], xt_d[:, :, 1536:2048])

            qt_sb = consts.tile([128, 2, S], F16)
            kt_sb = consts.tile([128, 2, S], F16)
            vaug = consts.tile([128, NT, HPG * 65], F16)
            zt_sb = consts.tile([128, 2, S], F16)
            nc.vector.memset(vaug[:], 1.0)
            mask2 = consts.tile([128, 2, 128], F16)
            nc.vector.tensor_copy(mask2[:, 0, :], mask_t[:])
            nc.vector.tensor_copy(mask2[:, 1, :], mask_t[:])

            # ---- pools ----
            stp = ctx.enter_context(tc.tile_pool(name="stp", bufs=2, space="PSUM"))
            obp = ctx.enter_context(tc.tile_pool(name="obp", bufs=3, space="PSUM"))
            pp = ctx.enter_context(tc.tile_pool(name="pp", bufs=1, space="PSUM"))
            ptp = ctx.enter_context(tc.tile_pool(name="ptp", bufs=10))
            rp = ctx.enter_context(tc.tile_pool(name="rope", bufs=8))
            smp = ctx.enter_context(tc.tile_pool(name="smp", bufs=4))
            bcp = ctx.enter_context(tc.tile_pool(name="bcp", bufs=4))
            yp = ctx.enter_context(tc.tile_pool(name="ysb", bufs=4))

            # HAM warm-up: ~13us of dummy matmuls during the DMA window so
            # the PE clock gate is at 8/8 when real work starts; the final
            # exp read also pulls in the ACT exp table set early.
            dummy_ps = stp.tile([128, 2, 512], F32, tag="st", name="dummy_ps")
            for _ in range(16):
                nc.tensor.matmul(
                    dummy_ps[:, 0, :],
                    lhsT=dummy_sb[:, 0:128],
                    rhs=dummy_sb[:],
                    start=True,
                    stop=True,
                    skip_group_check=True,
                )
            warm2 = consts.tile([1, 2], F16)
            nc.scalar.activation(warm2[:], dummy_ps[0:1, 0, 0:2], Exp, scale=1.0)

            fillers = deque()

            def weave(n):
                for _ in range(n):
                    if fillers:
                        fillers.popleft()()

            # ================= projection machinery =================
            def make_proj_chunk(wsb, dest, c, t):
                def emit():
                    lo, hi = t * 512, (t + 1) * 512
                    psq = pp.tile([128, 512], F32, tag="proj", name=f"ps_{c}_{t}")
                    for k in range(8):
                        nc.tensor.matmul(
                            psq[:],
                            lhsT=wsb[:, k, c * 128:(c + 1) * 128],
                            rhs=xt_sb[:, k, lo:hi],
                            start=(k == 0),
                            stop=(k == 7),
                        )
                    qc = rp.tile([128, 512], F16, tag="rope")
                    nc.vector.tensor_copy(qc[:], psq[:])  # fp32->fp16 cast
                    t1 = rp.tile([128, 512], F16, tag="rope")
                    nc.vector.tensor_mul(t1[:], qc[:], cos_t[:, lo:hi])
                    rot = rp.tile([128, 512], F16, tag="rope")
                    for qq in range(4):
                        srcp = (qq ^ 1) * 32
                        nc.vector.tensor_copy(
                            rot[qq * 32:(qq + 1) * 32, :], qc[srcp:srcp + 32, :]
                        )
                    t2 = rp.tile([128, 512], F16, tag="rope")
                    nc.vector.tensor_mul(t2[:], rot[:], sin_t[:, lo:hi])
                    nc.vector.tensor_add(dest[:, c, lo:hi], t1[:], t2[:])
                return emit

            def make_vproj_block(tb):
                def emit():
                    psv = pp.tile([128, 512], F32, tag="proj", name=f"psv_{tb}")
                    for k in range(8):
                        nc.tensor.matmul(
                            psv[:, 0:FG],
                            lhsT=xt_sb[:, k, tb * 128:(tb + 1) * 128],
                            rhs=wv_sb[:, k, :],
                            start=(k == 0),
                            stop=(k == 7),
                        )
                    nc.vector.tensor_copy(
                        vaug[:, tb, :].rearrange("p (h c) -> p h c", c=65)[:, :, 0:64],
                        psv[:, 0:FG].rearrange("p (h d) -> p h d", d=64),
                    )
                return emit

            # ================= O-projection machinery =================
            yt_r = yt_d[:]

            def make_oproj_chunk(dchunk, scol, cast_act=False, pool=None):
                def emit():
                    psy = (pool or pp).tile(
                        [128, 512], F32,
                        tag="ob" if pool is not None else "proj",
                        name=f"psy_{dchunk}_{scol}",
                    )
                    for c2 in range(2):
                        nc.tensor.matmul(
                            psy[:],
                            lhsT=wo_sb[:, c2, dchunk * 128:(dchunk + 1) * 128],
                            rhs=zt_sb[:, c2, scol:scol + 512],
                            start=(c2 == 0),
                            stop=(c2 == 1),
                        )
                    ytile = yp.tile(
                        [128, 512], F16, tag="y", name=f"y_{dchunk}_{scol}"
                    )
                    if cast_act:
                        nc.scalar.copy(ytile[:], psy[:])
                    else:
                        nc.vector.tensor_copy(ytile[:], psy[:])
                    nc.sync.dma_start(yt_r[:, dchunk, scol:scol + 512], ytile[:])
                return emit

            def owindow_chunks(scol_base, cast_act=False):
                out = []
                for dchunk in range(8):
                    out.append(make_oproj_chunk(dchunk, scol_base, cast_act))
                    out.append(make_oproj_chunk(dchunk, scol_base + 512, cast_act))
                return out

            # ================= attention unit =================
            # unit (qt, p): heads hh=0 rows 0:64, hh=1 rows 64:128, slot p.
            def emit_unit(qt, p, nf_per_j):
                qlo = qt * QTILE
                jmax = (qlo + QTILE) // 128
                pts = {}
                obs = {}
                for hh in range(2):
                    obs[hh] = obp.tile(
                        [65, QTILE], F32, tag="ob", name=f"ob_{qt}_{p}_{hh}"
                    )

                def pv(j):
                    pt, a0, w = pts.pop(j)
                    for hh in range(2):
                        hg = p * 2 + hh
                        nc.tensor.matmul(
                            obs[hh][:, a0:a0 + w],
                            lhsT=vaug[:, j, hg * 65:(hg + 1) * 65],
                            rhs=pt[:, hh, 0:w],
                            start=(j == 0),
                            stop=(j == jmax - 1),
                            skip_group_check=True,
                        )

                def normalize(hh):
                    cols = slice(qlo, qlo + QTILE)
                    lrow = smp.tile([1, QTILE], F32, tag="lrow")
                    nc.vector.tensor_copy(lrow[:], obs[hh][64:65, :])
                    rcp = smp.tile([1, QTILE], F32, tag="rcp")
                    nc.vector.reciprocal_approx_fast(rcp[:], lrow[:])
                    bc = bcp.tile([64, QTILE], F32, tag="bc")
                    nc.gpsimd.partition_broadcast(bc[:], rcp[:])
                    nc.vector.tensor_mul(
                        zt_sb[hh * 64:(hh + 1) * 64, p, cols], obs[hh][0:64, :], bc[:]
                    )

                for j in range(jmax):
                    qs = max(qlo, j * 128)
                    a0 = qs - qlo
                    w = QTILE - a0
                    st = stp.tile([128, 2, 512], F32, tag="st", name=f"st_{qt}_{p}_{j}")
                    for hh in range(2):
                        base = 64 * hh
                        nc.tensor.matmul(
                            st[:, hh, 0:w],
                            lhsT=kt_sb[base:base + 64, p, j * 128:(j + 1) * 128],
                            rhs=qt_sb[base:base + 64, p, qs:qs + w],
                            start=True,
                            stop=True,
                        )
                    pt = ptp.tile([128, 2, 512], F16, tag="pt")
                    nc.scalar.activation(pt[:, :, 0:w], st[:, :, 0:w], Exp, scale=0.125)
                    if j * 128 >= qlo:
                        nc.vector.tensor_mul(pt[:, :, 0:128], pt[:, :, 0:128], mask2[:])
                    pts[j] = (pt, a0, w)
                    if j >= 2:
                        pv(j - 2)
                    weave(nf_per_j)
                if jmax >= 2:
                    pv(jmax - 2)
                pv(jmax - 1)
                normalize(0)
                normalize(1)

            def owindow(qlo, cast_act=False, pool=None):
                return [make_oproj_chunk(d, qlo, cast_act, pool) for d in range(8)]

            # ================= schedule =================
            # prologue: just enough K/Q/V for (qt0, p0)
            make_proj_chunk(wk_sb, kt_sb, 0, 0)()
            make_proj_chunk(wq_sb, qt_sb, 0, 0)()
            make_vproj_block(0)()
            # filler order respects downstream deps (qtiles are 512 cols;
            # K chunk t covers k-blocks 4t..4t+3; V block tb = k-block tb)
            fillers.append(make_vproj_block(1))
            fillers.append(make_vproj_block(2))
            fillers.append(make_vproj_block(3))
            fillers.append(make_proj_chunk(wq_sb, qt_sb, 1, 0))   # qt0p1
            fillers.append(make_proj_chunk(wk_sb, kt_sb, 1, 0))
            fillers.append(make_proj_chunk(wk_sb, kt_sb, 0, 1))   # qt1
            fillers.append(make_proj_chunk(wq_sb, qt_sb, 0, 1))
            for tb in range(4, 8):
                fillers.append(make_vproj_block(tb))
            fillers.append(make_proj_chunk(wk_sb, kt_sb, 1, 1))
            fillers.append(make_proj_chunk(wq_sb, qt_sb, 1, 1))
            fillers.append(make_proj_chunk(wk_sb, kt_sb, 0, 2))   # qt2
            fillers.append(make_proj_chunk(wq_sb, qt_sb, 0, 2))
            for tb in range(8, 12):
                fillers.append(make_vproj_block(tb))
            fillers.append(make_proj_chunk(wk_sb, kt_sb, 1, 2))
            fillers.append(make_proj_chunk(wq_sb, qt_sb, 1, 2))
            fillers.append(make_proj_chunk(wk_sb, kt_sb, 0, 3))   # qt3
            fillers.append(make_proj_chunk(wq_sb, qt_sb, 0, 3))
            fillers.append(make_proj_chunk(wk_sb, kt_sb, 1, 3))
            fillers.append(make_proj_chunk(wq_sb, qt_sb, 1, 3))

            emit_unit(0, 0, nf_per_j=3)
            emit_unit(0, 1, nf_per_j=3)
            w0 = owindow(0)
            fillers.extend(w0[0:4])
            emit_unit(1, 0, nf_per_j=2)
            fillers.extend(w0[4:8])
            emit_unit(1, 1, nf_per_j=2)
            w1 = owindow(512)
            fillers.extend(w1[0:4])
            emit_unit(2, 0, nf_per_j=1)
            fillers.extend(w1[4:8])
            emit_unit(2, 1, nf_per_j=1)
            for tb in range(12, 16):
                fillers.append(make_vproj_block(tb))
            w2 = owindow(1024)
            fillers.extend(w2[0:4])
            emit_unit(3, 0, nf_per_j=1)
            fillers.extend(w2[4:8])
            emit_unit(3, 1, nf_per_j=1)
            # tail: leftovers + final O window (casts on ACT, idle then)
            while fillers:
                fillers.popleft()()
            for fn in owindow(1536, cast_act=True, pool=obp):
                fn()

    nc.compile()
    return nc


def _host_inputs(x, Wq, Wk, Wv, Wo, S):
    """Per-core input maps (host-side sharding + layout prep)."""
    # de-interleave RoPE pairs within each head: (2i, 2i+1) -> (i, i+32)
    perm = np.concatenate([np.arange(0, HEAD_DIM, 2), np.arange(1, HEAD_DIM, 2)])
    rp = (np.arange(HPG)[:, None] * HEAD_DIM + perm[None, :]).reshape(-1)

    half = HEAD_DIM // 2
    inv_freq = THETA ** (-np.arange(half, dtype=np.float64) * 2.0 / HEAD_DIM)
    ang = np.arange(S, dtype=np.float64)[None, :] * inv_freq[:, None]  # [32, S]
    cos32 = np.cos(ang)
    sin32 = np.sin(ang)
    cos128 = np.tile(cos32, (4, 1)).astype(np.float16)
    sins128 = np.concatenate([-sin32, sin32, -sin32, sin32], axis=0).astype(np.float16)
    mask = (np.arange(128)[None, :] >= np.arange(128)[:, None]).astype(np.float16)

    def part3(a, c):
        # [DIM_like, F] -> [128, c, F] partition-major dense
        return np.ascontiguousarray(
            a.reshape(c, 128, a.shape[1]).transpose(1, 0, 2)
        ).astype(np.float16)

    in_maps = []
    for core in range(B * GROUPS):
        b, g = divmod(core, GROUPS)
        sl = slice(g * FG, (g + 1) * FG)
        in_maps.append(
            dict(
                xt=part3(x[b].T.astype(np.float32), 8),
                wqt=part3(Wq[sl][rp].T, 8),
                wkt=part3(Wk[sl][rp].T, 8),
                wvt=part3(Wv[sl].T, 8),
                wot=part3(Wo[:, sl].T, 2),
                cos=cos128,
                sins=sins128,
                mask=mask,
            )
        )
    return in_maps


def _install_ntff_hook():
    """Provide antenv.axon_hooks if the image lacks it (NTFF profiling
    under axon; mirrors trn_agent_boot._ntff_profile_via_ctypes)."""
    try:
        from antenv.axon_hooks import get_axon_ntff_profile_hook  # noqa: F401
        return
    except ImportError:
        pass
    import contextlib
    import ctypes
    import types

    so_path = "/opt/axon/libaxon_pjrt.so"
    if not os.path.exists(so_path):
        return
    lib = ctypes.CDLL(so_path)
    if not hasattr(lib, "axon_start_nrt_profile"):
        return
    lib.axon_start_nrt_profile.argtypes = [
        ctypes.POINTER(ctypes.c_int64),
        ctypes.c_size_t,
    ]
    lib.axon_start_nrt_profile.restype = ctypes.c_int64
    lib.axon_stop_nrt_profile.argtypes = [ctypes.c_char_p]
    lib.axon_stop_nrt_profile.restype = ctypes.c_int64

    @contextlib.contextmanager
    def _hook(output_dir, device_ids):
        import jax

        jax.devices()
        if device_ids:
            ids = (ctypes.c_int64 * len(device_ids))(*device_ids)
            rc = lib.axon_start_nrt_profile(ids, len(device_ids))
        else:
            rc = lib.axon_start_nrt_profile(None, 0)
        if rc != 0:
            raise RuntimeError(f"axon_start_nrt_profile rc={rc}")
        try:
            yield
        finally:
            n = lib.axon_stop_nrt_profile(str(output_dir).encode())
            print(f"profile: {n} file(s) written to {output_dir}")

    mod = types.ModuleType("antenv.axon_hooks")
    _state = {"hook": _hook}
    mod.get_axon_ntff_profile_hook = lambda: _state["hook"]
    mod.set_axon_ntff_profile_hook = lambda h: _state.__setitem__("hook", h)
    import antenv

    antenv.axon_hooks = mod
    sys.modules["antenv.axon_hooks"] = mod


_NC_CACHE = {}


def _get_nc(S):
    if S not in _NC_CACHE:
        _NC_CACHE[S] = build_program(S)
    return _NC_CACHE[S]


def kernel(x, Wq, Wk, Wv, Wo, _trace=False, _tmpdir=None):
    x = np.asarray(x, dtype=np.float32)
    Wq = np.asarray(Wq, dtype=np.float32)
    Wk = np.asarray(Wk, dtype=np.float32)
    Wv = np.asarray(Wv, dtype=np.float32)
    Wo = np.asarray(Wo, dtype=np.float32)
    S = x.shape[1]

    if _trace:
        _install_ntff_hook()
    nc = _get_nc(S)
    in_maps = _host_inputs(x, Wq, Wk, Wv, Wo, S)
    res = run_bass_kernel_spmd(
        nc, in_maps, core_ids=list(range(8)), trace=_trace, tmpdir=_tmpdir
    )
    # yt arrives as [128, 8, S]; reassemble to [DIM, S] then transpose
    yts = [
        res.results[c]["yt"].astype(np.float32).transpose(1, 0, 2).reshape(DIM, S)
        for c in range(8)
    ]
    y = np.stack(
        [sum(yts[b * GROUPS + g] for g in range(GROUPS)).T for b in range(B)]
    ).astype(np.float32)
    if _trace:
        kernel.last_results = res
    return y


# revision 26
# speedup vs baseline: 1.4521x; 1.1982x over previous
"""Trainium2 Bass kernel for causal multi-head attention with RoPE.

Problem: B=2, S=2048, DIM=1024, 16 heads, head_dim=64.
  q = x @ Wq.T ; k = x @ Wk.T ; v = x @ Wv.T        (torch Linear convention)
  q, k = rope(q), rope(k)                            (Llama interleaved pairs)
  y = softmax(causal(q k^T / 8)) v @ Wo.T

Sharding (8 cores): data-parallel over batch (2) x tensor-parallel over
head groups (4 heads per core).  Wq/Wk/Wv row-sharded, Wo column-sharded;
the host sums the 4 partial outputs per batch.

v2: one fused pipeline instead of serial phases.  The exp stream on
ScalarE (~105us incl. per-instr overhead) and the PE column stream
(~112us) are the two floors; the program interleaves them so neither
engine idles: attention scores+exp start ~8us in, and all projection /
V-projection / O-projection matmuls are woven into the attention loop
as PE filler, which also keeps the PE busy enough that the HAM clock
gate stays at 8/8 (the baseline lost ~35us to 1.2GHz throttling).

PSUM budget (8 banks x 2KB):
  ST   1 tag x 2 bufs x [128,512] f32 = 2 banks  (score^T chunks; the
       single-buffering per head paces the PE to the ACT exp stream)
  OB   2 bufs x [65,1024] f32         = 4 banks  (PV accum + ones-row
       denominator, one per head)
  pp   2 bufs x [128,512] f32         = 2 banks  (proj + O-proj chunks)

Per (qtile, pair) unit j-loop: scores h0/h1 issued adjacently (PE row
groups 0 and 64 -> concurrent row-tiled matmuls), exp per 512-col chunk
on ACT, inline PV at lag 2, normalization per 512-col half as soon as
its j's complete (reciprocal_approx_fast on the PSUM ones-row, GPSIMD
partition-broadcast, one DVE multiply into Z^T fp16).  O-projection
chunks chase the normalized halves and DMA out fp16.
"""

import os
import sys

sys.path.insert(0, "/opt/trn_rl_repo")

from collections import deque

import numpy as np

import concourse.bass as bass
import concourse.mybir as mybir
import concourse.tile as tile
from concourse import bacc
from concourse.bass_utils import run_bass_kernel_spmd

F16 = mybir.dt.float16
F32 = mybir.dt.float32

DIM = 1024
NUM_HEADS = 16
HEAD_DIM = 64
B = 2
GROUPS = 4                   # head groups (tensor parallel)
HPG = NUM_HEADS // GROUPS    # heads per group = 4
FG = HPG * HEAD_DIM          # features per group = 256
THETA = 10000.0


def build_program(S=2048):
    from contextlib import ExitStack

    nc = bacc.Bacc(None, target_bir_lowering=False)
    NT = S // 128                 # token blocks
    QTILE = 512
    NQT = S // QTILE

    # all inputs host-prearranged to [partition, ...] dense layouts so every
    # DMA is a single-stage contiguous transfer (no on-the-fly rearrange)
    xt_d = nc.declare_dram_parameter("xt", [128, 8, S], F16, isOutput=False)
    wq_d = nc.declare_dram_parameter("wqt", [128, 8, FG], F16, isOutput=False)
    wk_d = nc.declare_dram_parameter("wkt", [128, 8, FG], F16, isOutput=False)
    wv_d = nc.declare_dram_parameter("wvt", [128, 8, FG], F16, isOutput=False)
    wo_d = nc.declare_dram_parameter("wot", [128, 2, DIM], F16, isOutput=False)
    cos_d = nc.declare_dram_parameter("cos", [128, S], F16, isOutput=False)
    sin_d = nc.declare_dram_parameter("sins", [128, S], F16, isOutput=False)
    mask_d = nc.declare_dram_parameter("mask", [128, 128], F16, isOutput=False)
    # fp16 partial output [partition, dchunk, token]; host reassembles
    yt_d = nc.declare_dram_parameter("yt", [128, 8, S], F16, isOutput=True)

    Exp = mybir.ActivationFunctionType.Exp

    with tile.TileContext(nc) as tc:
        with ExitStack() as ctx:
            consts = ctx.enter_context(tc.tile_pool(name="consts", bufs=1))

            # ---- constants / persistent SBUF ----
            dummy_sb = consts.tile([128, 512], F16)
            nc.vector.memset(dummy_sb[:], 1.0)
            wk_sb = consts.tile([128, 8, FG], F16)
            nc.sync.dma_start(wk_sb[:, 0:2, :], wk_d[:, 0:2, :])
            xt_sb = consts.tile([128, 8, S], F16)
            nc.sync.dma_start(xt_sb[:, :, 0:512], xt_d[:, :, 0:512])
            nc.sync.dma_start(wk_sb[:, 2:8, :], wk_d[:, 2:8, :])
            cos_t = consts.tile([128, S], F16)
            nc.sync.dma_start(cos_t[:], cos_d[:])
            sin_t = consts.tile([128, S], F16)
            nc.sync.dma_start(sin_t[:], sin_d[:])
            wq_sb = consts.tile([128, 8, FG], F16)
            nc.sync.dma_start(wq_sb[:], wq_d[:])
            mask_t = consts.tile([128, 128], F16)
            nc.sync.dma_start(mask_t[:], mask_d[:])
            wv_sb = consts.tile([128, 8, FG], F16)
            nc.sync.dma_start(wv_sb[:], wv_d[:])
            for t in range(1, 3):
                c0, c1 = t * 512, (t + 1) * 512
                nc.sync.dma_start(xt_sb[:, :, c0:c1], xt_d[:, :, c0:c1])
            wo_sb = consts.tile([128, 2, DIM], F16)
            nc.sync.dma_start(wo_sb[:], wo_d[:])
            nc.sync.dma_start(xt_sb[:, :, 1536:2048], xt_d[:, :, 1536:2048])

            qt_sb = consts.tile([128, 2, S], F16)
            kt_sb = consts.tile([128, 2, S], F16)
            vaug = consts.tile([128, NT, HPG * 65], F16)
            zt_sb = consts.tile([128, 2, S], F16)
            nc.vector.memset(vaug[:], 1.0)
            mask2 = consts.tile([128, 2, 128], F16)
            nc.vector.tensor_copy(mask2[:, 0, :], mask_t[:])
            nc.vector.tensor_copy(mask2[:, 1, :], mask_t[:])

            # ---- pools ----
            stp = ctx.enter_context(tc.tile_pool(name="stp", bufs=2, space="PSUM"))
            obp = ctx.enter_context(tc.tile_pool(name="obp", bufs=3, space="PSUM"))
            pp = ctx.enter_context(tc.tile_pool(name="pp", bufs=1, space="PSUM"))
            ptp = ctx.enter_context(tc.tile_pool(name="ptp", bufs=10))
            rp = ctx.enter_context(tc.tile_pool(name="rope", bufs=8))
            smp = ctx.enter_context(tc.tile_pool(name="smp", bufs=4))
            bcp = ctx.enter_context(tc.tile_pool(name="bcp", bufs=4))
            yp = ctx.enter_context(tc.tile_pool(name="ysb", bufs=4))

            # HAM warm-up: ~13us of dummy matmuls during the DMA window so
            # the PE clock gate is at 8/8 when real work starts; the final
            # exp read also pulls in the ACT exp table set early.
            dummy_ps = stp.tile([128, 2, 512], F32, tag="st", name="dummy_ps")
            for _ in range(36):
                nc.tensor.matmul(
                    dummy_ps[:, 0, :],
                    lhsT=dummy_sb[:, 0:128],
                    rhs=dummy_sb[:],
                    start=True,
                    stop=True,
                    skip_group_check=True,
                )
            warm2 = consts.tile([1, 2], F16)
            nc.scalar.activation(warm2[:], dummy_ps[0:1, 0, 0:2], Exp, scale=1.0)

            fillers = deque()

            def weave(n):
                for _ in range(n):
                    if fillers:
                        fillers.popleft()()

            # ================= projection machinery =================
            def make_proj_chunk(wsb, dest, c, t):
                def emit():
                    lo, hi = t * 512, (t + 1) * 512
                    psq = pp.tile([128, 512], F32, tag="proj", name=f"ps_{c}_{t}")
                    for k in range(8):
                        nc.tensor.matmul(
                            psq[:],
                            lhsT=wsb[:, k, c * 128:(c + 1) * 128],
                            rhs=xt_sb[:, k, lo:hi],
                            start=(k == 0),
                            stop=(k == 7),
                        )
                    qc = rp.tile([128, 512], F16, tag="rope")
                    nc.vector.tensor_copy(qc[:], psq[:])  # fp32->fp16 cast
                    t1 = rp.tile([128, 512], F16, tag="rope")
                    nc.vector.tensor_mul(t1[:], qc[:], cos_t[:, lo:hi])
                    rot = rp.tile([128, 512], F16, tag="rope")
                    for qq in range(4):
                        srcp = (qq ^ 1) * 32
                        nc.vector.tensor_copy(
                            rot[qq * 32:(qq + 1) * 32, :], qc[srcp:srcp + 32, :]
                        )
                    t2 = rp.tile([128, 512], F16, tag="rope")
                    nc.vector.tensor_mul(t2[:], rot[:], sin_t[:, lo:hi])
                    nc.vector.tensor_add(dest[:, c, lo:hi], t1[:], t2[:])
                return emit

            def make_vproj_block(tb):
                def emit():
                    psv = pp.tile([128, 512], F32, tag="proj", name=f"psv_{tb}")
                    for k in range(8):
                        nc.tensor.matmul(
                            psv[:, 0:FG],
                            lhsT=xt_sb[:, k, tb * 128:(tb + 1) * 128],
                            rhs=wv_sb[:, k, :],
                            start=(k == 0),
                            stop=(k == 7),
                        )
                    nc.vector.tensor_copy(
                        vaug[:, tb, :].rearrange("p (h c) -> p h c", c=65)[:, :, 0:64],
                        psv[:, 0:FG].rearrange("p (h d) -> p h d", d=64),
                    )
                return emit

            # ================= O-projection machinery =================
            yt_r = yt_d[:]

            def make_oproj_chunk(dchunk, scol, cast_act=False, pool=None):
                def emit():
                    psy = (pool or pp).tile(
                        [128, 512], F32,
                        tag="ob" if pool is not None else "proj",
                        name=f"psy_{dchunk}_{scol}",
                    )
                    for c2 in range(2):
                        nc.tensor.matmul(
                            psy[:],
                            lhsT=wo_sb[:, c2, dchunk * 128:(dchunk + 1) * 128],
                            rhs=zt_sb[:, c2, scol:scol + 512],
                            start=(c2 == 0),
                            stop=(c2 == 1),
                        )
                    ytile = yp.tile(
                        [128, 512], F16, tag="y", name=f"y_{dchunk}_{scol}"
                    )
                    if cast_act:
                        nc.scalar.copy(ytile[:], psy[:])
                    else:
                        nc.vector.tensor_copy(ytile[:], psy[:])
                    nc.sync.dma_start(yt_r[:, dchunk, scol:scol + 512], ytile[:])
                return emit

            def owindow_chunks(scol_base, cast_act=False):
                out = []
                for dchunk in range(8):
                    out.append(make_oproj_chunk(dchunk, scol_base, cast_act))
                    out.append(make_oproj_chunk(dchunk, scol_base + 512, cast_act))
                return out

            # ================= attention unit =================
            # unit (qt, p): heads hh=0 rows 0:64, hh=1 rows 64:128, slot p.
            def emit_unit(qt, p, nf_per_j):
                qlo = qt * QTILE
                jmax = (qlo + QTILE) // 128
                pts = {}
                obs = {}
                for hh in range(2):
                    obs[hh] = obp.tile(
                        [65, QTILE], F32, tag="ob", name=f"ob_{qt}_{p}_{hh}"
                    )

                def pv(j):
                    pt, a0, w = pts.pop(j)
                    for hh in range(2):
                        hg = p * 2 + hh
                        nc.tensor.matmul(
                            obs[hh][:, a0:a0 + w],
                            lhsT=vaug[:, j, hg * 65:(hg + 1) * 65],
                            rhs=pt[:, hh, 0:w],
                            start=(j == 0),
                            stop=(j == jmax - 1),
                            skip_group_check=True,
                        )

                def normalize(hh):
                    cols = slice(qlo, qlo + QTILE)
                    lrow = smp.tile([1, QTILE], F32, tag="lrow")
                    nc.vector.tensor_copy(lrow[:], obs[hh][64:65, :])
                    rcp = smp.tile([1, QTILE], F32, tag="rcp")
                    nc.vector.reciprocal_approx_fast(rcp[:], lrow[:])
                    bc = bcp.tile([64, QTILE], F32, tag="bc")
                    nc.gpsimd.partition_broadcast(bc[:], rcp[:])
                    nc.vector.tensor_mul(
                        zt_sb[hh * 64:(hh + 1) * 64, p, cols], obs[hh][0:64, :], bc[:]
                    )

                for j in range(jmax):
                    qs = max(qlo, j * 128)
                    a0 = qs - qlo
                    w = QTILE - a0
                    st = stp.tile([128, 2, 512], F32, tag="st", name=f"st_{qt}_{p}_{j}")
                    for hh in range(2):
                        base = 64 * hh
                        nc.tensor.matmul(
                            st[:, hh, 0:w],
                            lhsT=kt_sb[base:base + 64, p, j * 128:(j + 1) * 128],
                            rhs=qt_sb[base:base + 64, p, qs:qs + w],
                            start=True,
                            stop=True,
                        )
                    pt = ptp.tile([128, 2, 512], F16, tag="pt")
                    nc.scalar.activation(pt[:, :, 0:w], st[:, :, 0:w], Exp, scale=0.125)
                    if j * 128 >= qlo:
                        nc.vector.tensor_mul(pt[:, :, 0:128], pt[:, :, 0:128], mask2[:])
                    pts[j] = (pt, a0, w)
                    if j >= 2:
                        pv(j - 2)
                    weave(nf_per_j)
                if jmax >= 2:
                    pv(jmax - 2)
                pv(jmax - 1)
                normalize(0)
                normalize(1)

            def owindow(qlo, cast_act=False, pool=None):
                return [make_oproj_chunk(d, qlo, cast_act, pool) for d in range(8)]

            # ================= schedule =================
            # prologue: just enough K/Q/V for (qt0, p0)
            make_proj_chunk(wk_sb, kt_sb, 0, 0)()
            make_proj_chunk(wq_sb, qt_sb, 0, 0)()
            make_vproj_block(0)()
            # filler order respects downstream deps (qtiles are 512 cols;
            # K chunk t covers k-blocks 4t..4t+3; V block tb = k-block tb)
            fillers.append(make_vproj_block(1))
            fillers.append(make_vproj_block(2))
            fillers.append(make_vproj_block(3))
            fillers.append(make_proj_chunk(wq_sb, qt_sb, 1, 0))   # qt0p1
            fillers.append(make_proj_chunk(wk_sb, kt_sb, 1, 0))
            fillers.append(make_proj_chunk(wk_sb, kt_sb, 0, 1))   # qt1
            fillers.append(make_proj_chunk(wq_sb, qt_sb, 0, 1))
            for tb in range(4, 8):
                fillers.append(make_vproj_block(tb))
            fillers.append(make_proj_chunk(wk_sb, kt_sb, 1, 1))
            fillers.append(make_proj_chunk(wq_sb, qt_sb, 1, 1))
            fillers.append(make_proj_chunk(wk_sb, kt_sb, 0, 2))   # qt2
            fillers.append(make_proj_chunk(wq_sb, qt_sb, 0, 2))
            for tb in range(8, 12):
                fillers.append(make_vproj_block(tb))
            fillers.append(make_proj_chunk(wk_sb, kt_sb, 1, 2))
            fillers.append(make_proj_chunk(wq_sb, qt_sb, 1, 2))
            fillers.append(make_proj_chunk(wk_sb, kt_sb, 0, 3))   # qt3
            fillers.append(make_proj_chunk(wq_sb, qt_sb, 0, 3))
            fillers.append(make_proj_chunk(wk_sb, kt_sb, 1, 3))
            fillers.append(make_proj_chunk(wq_sb, qt_sb, 1, 3))

            emit_unit(0, 0, nf_per_j=3)
            emit_unit(0, 1, nf_per_j=3)
            w0 = owindow(0)
            fillers.extend(w0[0:4])
            emit_unit(1, 0, nf_per_j=2)
            fillers.extend(w0[4:8])
            emit_unit(1, 1, nf_per_j=2)
            w1 = owindow(512)
            fillers.extend(w1[0:4])
            emit_unit(2, 0, nf_per_j=1)
            fillers.extend(w1[4:8])
            emit_unit(2, 1, nf_per_j=1)
            for tb in range(12, 16):
                fillers.append(make_vproj_block(tb))
            w2 = owindow(1024)
            fillers.extend(w2[0:4])
            emit_unit(3, 0, nf_per_j=1)
            fillers.extend(w2[4:8])
            emit_unit(3, 1, nf_per_j=1)
            # tail: leftovers + final O window (casts on ACT, idle then)
            while fillers:
                fillers.popleft()()
            for fn in owindow(1536, cast_act=True, pool=obp):
                fn()

    nc.compile()
    return nc


def _host_inputs(x, Wq, Wk, Wv, Wo, S):
    """Per-core input maps (host-side sharding + layout prep)."""
    # de-interleave RoPE pairs within each head: (2i, 2i+1) -> (i, i+32)
    perm = np.concatenate([np.arange(0, HEAD_DIM, 2), np.arange(1, HEAD_DIM, 2)])
    rp = (np.arange(HPG)[:, None] * HEAD_DIM + perm[None, :]).reshape(-1)

    half = HEAD_DIM // 2
    inv_freq = THETA ** (-np.arange(half, dtype=np.float64) * 2.0 / HEAD_DIM)
    ang = np.arange(S, dtype=np.float64)[None, :] * inv_freq[:, None]  # [32, S]
    cos32 = np.cos(ang)
    sin32 = np.sin(ang)
    cos128 = np.tile(cos32, (4, 1)).astype(np.float16)
    sins128 = np.concatenate([-sin32, sin32, -sin32, sin32], axis=0).astype(np.float16)
    mask = (np.arange(128)[None, :] >= np.arange(128)[:, None]).astype(np.float16)

    def part3(a, c):
        # [DIM_like, F] -> [128, c, F] partition-major dense
        return np.ascontiguousarray(
            a.reshape(c, 128, a.shape[1]).transpose(1, 0, 2)
        ).astype(np.float16)

    in_maps = []
    for core in range(B * GROUPS):
        b, g = divmod(core, GROUPS)
        sl = slice(g * FG, (g + 1) * FG)
        in_maps.append(
            dict(
                xt=part3(x[b].T.astype(np.float32), 8),
                wqt=part3(Wq[sl][rp].T, 8),
                wkt=part3(Wk[sl][rp].T, 8),
                wvt=part3(Wv[sl].T, 8),
                wot=part3(Wo[:, sl].T, 2),
                cos=cos128,
                sins=sins128,
                mask=mask,
            )
        )
    return in_maps


def _install_ntff_hook():
    """Provide antenv.axon_hooks if the image lacks it (NTFF profiling
    under axon; mirrors trn_agent_boot._ntff_profile_via_ctypes)."""
    try:
        from antenv.axon_hooks import get_axon_ntff_profile_hook  # noqa: F401
        return
    except ImportError:
        pass
    import contextlib
    import ctypes
    import types

    so_path = "/opt/axon/libaxon_pjrt.so"
    if not os.path.exists(so_path):
        return
    lib = ctypes.CDLL(so_path)
    if not hasattr(lib, "axon_start_nrt_profile"):
        return
    lib.axon_start_nrt_profile.argtypes = [
        ctypes.POINTER(ctypes.c_int64),
        ctypes.c_size_t,
    ]
    lib.axon_start_nrt_profile.restype = ctypes.c_int64
    lib.axon_stop_nrt_profile.argtypes = [ctypes.c_char_p]
    lib.axon_stop_nrt_profile.restype = ctypes.c_int64

    @contextlib.contextmanager
    def _hook(output_dir, device_ids):
        import jax

        jax.devices()
        if device_ids:
            ids = (ctypes.c_int64 * len(device_ids))(*device_ids)
            rc = lib.axon_start_nrt_profile(ids, len(device_ids))
        else:
            rc = lib.axon_start_nrt_profile(None, 0)
        if rc != 0:
            raise RuntimeError(f"axon_start_nrt_profile rc={rc}")
        try:
            yield
        finally:
            n = lib.axon_stop_nrt_profile(str(output_dir).encode())
            print(f"profile: {n} file(s) written to {output_dir}")

    mod = types.ModuleType("antenv.axon_hooks")
    _state = {"hook": _hook}
    mod.get_axon_ntff_profile_hook = lambda: _state["hook"]
    mod.set_axon_ntff_profile_hook = lambda h: _state.__setitem__("hook", h)
    import antenv

    antenv.axon_hooks = mod
    sys.modules["antenv.axon_hooks"] = mod


_NC_CACHE = {}


def _get_nc(S):
    if S not in _NC_CACHE:
        _NC_CACHE[S] = build_program(S)
    return _NC_CACHE[S]


def kernel(x, Wq, Wk, Wv, Wo, _trace=False, _tmpdir=None):
    x = np.asarray(x, dtype=np.float32)
    Wq = np.asarray(Wq, dtype=np.float32)
    Wk = np.asarray(Wk, dtype=np.float32)
    Wv = np.asarray(Wv, dtype=np.float32)
    Wo = np.asarray(Wo, dtype=np.float32)
    S = x.shape[1]

    if _trace:
        _install_ntff_hook()
    nc = _get_nc(S)
    in_maps = _host_inputs(x, Wq, Wk, Wv, Wo, S)
    res = run_bass_kernel_spmd(
        nc, in_maps, core_ids=list(range(8)), trace=_trace, tmpdir=_tmpdir
    )
    # yt arrives as [128, 8, S]; reassemble to [DIM, S] then transpose
    yts = [
        res.results[c]["yt"].astype(np.float32).transpose(1, 0, 2).reshape(DIM, S)
        for c in range(8)
    ]
    y = np.stack(
        [sum(yts[b * GROUPS + g] for g in range(GROUPS)).T for b in range(B)]
    ).astype(np.float32)
    if _trace:
        kernel.last_results = res
    return y
